# revision 2
# baseline (speedup 1.0000x reference)
"""Bispectrum on S1xS1 — Trainium2 Bass kernel (fp16 pipeline).

Full-input contract: kernel(x) with x (2, 64, 64) float32 returns
B (2, 4096, 4096) complex64 where, with X = fft2(x),
  B[b, (i,j), (p,q)] = X[b,i,j] * X[b,p,q] * conj(X[b,(i+p)%64,(j+q)%64]).

x is real, so X[-k,-l] = conj(X[k,l]) and B[rho(r), rho(c)] = conj(B[r,c])
with rho negating both frequency components. The device computes only rows
i in 0..33 (53% of the output); the host mirrors i in 34..63 by conjugation.

Sharding: each of the 8 cores computes ALL device rows for a 512-column
slice (p in [8k, 8k+8)) of both batches. Per-core column offsets are folded
into per-core DFT-matrix inputs (spectrum row-rotated by 8k), so the SPMD
program has no core-dependent access patterns.

The harness gate is rel_err < 2e-2 (normalized max error), so the
elementwise complex-multiply stage runs in fp16:
  - 64-pt DFTs on PE via host-passed DFT matrices (fp32, exact-ish)
  - rank-2 fp32r PE matmuls build ur/ui outer-product components in PSUM
  - Act copies PSUM -> SBUF fp16 with scale 2^-7 (enables DVE 2x mode)
  - circulant stacks cr/ci held fp16 (scaled 2^-6) via sliding-window DMA
  - DVE does 4 fp16 tensor_mul (2x mode) + the re-add; GpSimd does the
    im-subtract; output is planar [re(512) | im(512)] fp16 rows
  - host unscales by 2^13 and interleaves to complex64
"""

import os
import sys

for _p in ("/opt/trn_rl_repo", "/opt/pypackages"):
    if _p not in sys.path:
        sys.path.insert(0, _p)

import numpy as np

M = 64
MN = M * M
NCORES = 8
NI = 34                 # i-values computed on device (0..33)
GL = NI // 2            # 17 row-pair blocks per batch
DEV_ROWS = NI * M       # 2176 rows per batch
COLS = MN // NCORES     # 512 columns per core
VSLOTS = 40             # circulant stack w-slots: v = 2*gl + pl <= 39
XDD_ROWS = VSLOTS + 1   # v + s <= 40

SCALE_U = 2.0 ** -7     # applied to ur/ui on the PSUM->SBUF fp16 copy
SCALE_C = 2.0 ** -6     # applied to the circulant stack planes
HOST_SCALE = 2.0 ** 13  # undoes SCALE_U * SCALE_C on the host

_CACHE = {}


def _build_nc():
    import concourse.bass as bass
    import concourse.bacc as bacc
    import concourse.mybir as mybir
    from concourse.tile import TileContext

    f32 = mybir.dt.float32
    f16 = mybir.dt.float16
    f32r = mybir.dt.float32r
    nc = bacc.Bacc("TRN2")

    x = nc.declare_dram_parameter("x", [2, M, M], f32, isOutput=False)
    fr = nc.declare_dram_parameter("fr", [M, M], f32, isOutput=False)
    fi = nc.declare_dram_parameter("fi", [M, M], f32, isOutput=False)
    fin = nc.declare_dram_parameter("fin", [M, M], f32, isOutput=False)
    frr = nc.declare_dram_parameter("frr", [M, M], f32, isOutput=False)
    fir = nc.declare_dram_parameter("fir", [M, M], f32, isOutput=False)
    finr = nc.declare_dram_parameter("finr", [M, M], f32, isOutput=False)
    out = nc.declare_dram_parameter(
        "out", [2 * DEV_ROWS, 2 * COLS], f16, isOutput=True
    )

    # per-batch DRAM scratch
    dscratch = []
    for b in range(2):
        dscratch.append(
            dict(
                xa2_d=nc.dram_tensor(f"xa2_d{b}", [2, NI * M], f32),
                xb2_d=nc.dram_tensor(f"xb2_d{b}", [2, NI * M], f32),
                rhs2_d=nc.dram_tensor(f"rhs2_d{b}", [2, 8 * M], f32),
                xddr=nc.dram_tensor(f"xddr{b}", [XDD_ROWS, 128], f16),
                xddi=nc.dram_tensor(f"xddi{b}", [XDD_ROWS, 128], f16),
            )
        )

    with TileContext(nc) as tc:
        with (
            tc.tile_pool(name="const", bufs=1) as cp,
            tc.tile_pool(name="big", bufs=1) as bp,
            tc.tile_pool(name="u16", bufs=3) as up,
            tc.tile_pool(name="tmp", bufs=2) as tp,
            tc.tile_pool(name="chunkp", bufs=4) as kp,
        ):
          with tc.tile_pool(name="psum", bufs=2, space="PSUM") as pp:
              def sb64(src, tag):
                  t = cp.tile([M, M], f32, tag=tag)
                  nc.sync.dma_start(out=t, in_=src)
                  return t

              fr_sb = sb64(fr[:, :], "fr")
              fi_sb = sb64(fi[:, :], "fi")
              fin_sb = sb64(fin[:, :], "fin")
              frr_sb = sb64(frr[:, :], "frr")
              fir_sb = sb64(fir[:, :], "fir")
              finr_sb = sb64(finr[:, :], "finr")

              def mm2(lhs1, rhs1, lhs2, rhs2_, tagn):
                  ps = pp.tile([M, M], f32, tag="fft")
                  nc.tensor.matmul(ps[:, :], lhsT=lhs1, rhs=rhs1, start=True, stop=False)
                  nc.tensor.matmul(ps[:, :], lhsT=lhs2, rhs=rhs2_, start=False, stop=True)
                  sb = cp.tile([M, M], f32, tag=tagn)
                  nc.scalar.copy(sb, ps)
                  return sb

              def setup(b):
                  d = dscratch[b]
                  x_sb = sb64(x[b, :, :], f"x{b}")
                  # x^T via 32x32 stream-transpose blocks
                  xt_sb = cp.tile([M, M], f32, tag=f"xt{b}")
                  for bi_ in range(2):
                      for bj in range(2):
                          nc.vector.transpose(
                              xt_sb[bi_ * 32 : bi_ * 32 + 32, bj * 32 : bj * 32 + 32],
                              x_sb[bj * 32 : bj * 32 + 32, bi_ * 32 : bi_ * 32 + 32],
                          )
                  # stage 1: W = x @ F
                  wr_ps = pp.tile([M, M], f32, tag="fft")
                  nc.tensor.matmul(
                      wr_ps[:, :], lhsT=xt_sb, rhs=fr_sb, start=True, stop=True
                  )
                  wr_sb = cp.tile([M, M], f32, tag=f"wr{b}")
                  nc.scalar.copy(wr_sb, wr_ps)
                  wi_ps = pp.tile([M, M], f32, tag="fft")
                  nc.tensor.matmul(
                      wi_ps[:, :], lhsT=xt_sb, rhs=fi_sb, start=True, stop=True
                  )
                  wi_sb = cp.tile([M, M], f32, tag=f"wi{b}")
                  nc.scalar.copy(wi_sb, wi_ps)

                  # stage 2 unrotated (a-side rows) and rotated (b-side + stack)
                  xr_sb = mm2(fr_sb, wr_sb, fin_sb, wi_sb, f"xr{b}")
                  xi_sb = mm2(fr_sb, wi_sb, fi_sb, wr_sb, f"xi{b}")
                  xrr_sb = mm2(frr_sb, wr_sb, finr_sb, wi_sb, f"xrr{b}")
                  xri_sb = mm2(frr_sb, wi_sb, fir_sb, wr_sb, f"xri{b}")

                  # fp16 scaled rotated-spectrum planes for the stack source
                  xddr16 = cp.tile([XDD_ROWS, M], f16, tag=f"xddr16{b}")
                  nc.vector.tensor_scalar_mul(
                      xddr16, xrr_sb[0:XDD_ROWS, :], SCALE_C
                  )
                  xddi16 = cp.tile([XDD_ROWS, M], f16, tag=f"xddi16{b}")
                  nc.vector.tensor_scalar_mul(
                      xddi16, xri_sb[0:XDD_ROWS, :], SCALE_C
                  )
                  # doubled columns in DRAM (rows 0..XDD_ROWS all < 64: no wrap)
                  for (xdd, src_sb) in ((d["xddr"], xddr16), (d["xddi"], xddi16)):
                      nc.scalar.dma_start(out=xdd[:, 0:64], in_=src_sb)
                      nc.scalar.dma_start(out=xdd[:, 64:128], in_=src_sb)

                  # circulant stack: call[(s,j), (v,q)] = xdd[v+s, j+q], fp16
                  call_r = bp.tile([128, VSLOTS * 64], f16, tag=f"call_r{b}")
                  call_i = bp.tile([128, VSLOTS * 64], f16, tag=f"call_i{b}")
                  call_engs = [nc.sync, nc.scalar, nc.scalar, nc.sync]
                  for ci_, (callt, xdd, s) in enumerate(
                      (c, xx, s)
                      for (c, xx) in ((call_r, d["xddr"]), (call_i, d["xddi"]))
                      for s in range(2)
                  ):
                      dest = callt[s * 64 : (s + 1) * 64, :].rearrange(
                          "j (v q) -> j v q", v=VSLOTS
                      )
                      srcap = bass.AP(
                          tensor=xdd,
                          offset=s * 128,
                          ap=[[1, 64], [128, VSLOTS], [1, 64]],
                      )
                      call_engs[ci_].dma_start(out=dest, in_=srcap)

                  # a-side lhsT rows: xa2 = [xr, -xi], xb2 = [xi, xr]
                  xin_sb = cp.tile([NI, M], f32, tag=f"xin{b}")
                  nc.vector.tensor_scalar_mul(xin_sb, xi_sb[0:NI, :], -1.0)

                  def stack_write(dst, rows_src, nrows, eng):
                      for r, t in enumerate(rows_src):
                          eng.dma_start(
                              out=dst[r : r + 1, :].rearrange(
                                  "r (p f) -> (r p) f", p=nrows
                              ),
                              in_=t,
                          )

                  stack_write(
                      d["xa2_d"], [xr_sb[0:NI, :], xin_sb], NI, nc.sync
                  )
                  stack_write(
                      d["xb2_d"], [xi_sb[0:NI, :], xr_sb[0:NI, :]], NI, nc.scalar
                  )
                  stack_write(
                      d["rhs2_d"], [xrr_sb[0:8, :], xri_sb[0:8, :]], 8, nc.sync
                  )
                  xa = bp.tile([2, NI * M], f32, tag=f"xa{b}")
                  nc.sync.dma_start(out=xa, in_=d["xa2_d"][:, :])
                  xb = bp.tile([2, NI * M], f32, tag=f"xb{b}")
                  nc.scalar.dma_start(out=xb, in_=d["xb2_d"][:, :])
                  rhs2 = bp.tile([2, 8 * M], f32, tag=f"rhs2{b}")
                  nc.sync.dma_start(out=rhs2, in_=d["rhs2_d"][:, :])

                  return dict(xa=xa, xb=xb, rhs2=rhs2, cr=call_r, ci=call_i)

              def mainloop(b, t_):
                  for gl in range(GL):
                      v0 = 2 * gl
                      ur = pp.tile([128, COLS], f32, tag="ur", bufs=3)
                      ui = pp.tile([128, COLS], f32, tag="ui", bufs=3)
                      lsl = slice(gl * 128, gl * 128 + 128)
                      nc.tensor.matmul(
                          ur[:, :],
                          lhsT=t_["xa"][:, lsl].bitcast(f32r),
                          rhs=t_["rhs2"][:, :].bitcast(f32r),
                          start=True, stop=True,
                      )
                      nc.tensor.matmul(
                          ui[:, :],
                          lhsT=t_["xb"][:, lsl].bitcast(f32r),
                          rhs=t_["rhs2"][:, :].bitcast(f32r),
                          start=True, stop=True,
                      )
                      # scaled fp16 copies PSUM -> SBUF on Act
                      ur16 = up.tile([128, COLS], f16, tag="ur16")
                      nc.scalar.mul(ur16, ur, SCALE_U)
                      ui16 = up.tile([128, COLS], f16, tag="ui16")
                      nc.scalar.mul(ui16, ui, SCALE_U)

                      csl = slice(v0 * 64, v0 * 64 + COLS)
                      crw = t_["cr"][:, csl]
                      ciw = t_["ci"][:, csl]
                      m1 = tp.tile([128, COLS], f16, tag="m1")
                      m2 = tp.tile([128, COLS], f16, tag="m2")
                      m3 = tp.tile([128, COLS], f16, tag="m3")
                      m4 = tp.tile([128, COLS], f16, tag="m4")
                      nc.vector.tensor_mul(m1, ur16, crw)
                      nc.vector.tensor_mul(m2, ui16, ciw)
                      nc.vector.tensor_mul(m3, ui16, crw)
                      nc.vector.tensor_mul(m4, ur16, ciw)
                      chunk = kp.tile([128, 2 * COLS], f16, tag="chunk")
                      nc.vector.tensor_add(chunk[:, 0:COLS], m1, m2)
                      nc.gpsimd.tensor_sub(chunk[:, COLS : 2 * COLS], m3, m4)
                      row0 = b * DEV_ROWS + gl * 128
                      out_eng = nc.sync if (gl % 2 == 0) else nc.scalar
                      out_eng.dma_start(
                          out=out[row0 : row0 + 128, :], in_=chunk
                      )

              # interleave: batch-1 setup instructions are emitted after
              # batch-0 main loop so they overlap it on idle engines
              for b in range(2):
                  t_ = setup(b)
                  mainloop(b, t_)
    nc.compile()
    return nc


def _dft_consts():
    k = np.arange(M)
    ang = -2.0 * np.pi * np.outer(k, k) / M
    Fr = np.cos(ang).astype(np.float32)
    Fi = np.sin(ang).astype(np.float32)
    return Fr, Fi


def _in_maps(x):
    Fr, Fi = _dft_consts()
    FiN = np.ascontiguousarray(-Fi)
    maps = []
    for core in range(NCORES):
        rFr = np.roll(Fr, -core * 8, axis=0)
        rFi = np.roll(Fi, -core * 8, axis=0)
        maps.append(
            {
                "x": x,
                "fr": Fr,
                "fi": Fi,
                "fin": FiN,
                "frr": np.ascontiguousarray(rFr.T),
                "fir": np.ascontiguousarray(rFi.T),
                "finr": np.ascontiguousarray(-rFi.T),
            }
        )
    return maps


def _assemble(results):
    out = np.empty((2, MN, MN), dtype=np.complex64)
    for core in range(NCORES):
        blk = np.asarray(results[core]["out"])
        blk = blk.astype(np.float32).reshape(2, DEV_ROWS, 2, COLS)
        blk *= HOST_SCALE
        csl = slice(core * COLS, (core + 1) * COLS)
        out[:, 0:DEV_ROWS, csl].real = blk[:, :, 0, :]
        out[:, 0:DEV_ROWS, csl].imag = blk[:, :, 1, :]
    # Hermitian mirror: rows i in 34..63 from conj at negated indices
    idx = np.arange(MN)
    rho = ((M - idx // M) % M) * M + (M - idx % M) % M
    rho_r = rho[DEV_ROWS:]
    for b in range(2):
        out[b, DEV_ROWS:, :] = np.conj(out[b, rho_r, :][:, rho])
    return out


def kernel(x):
    from concourse.bass_utils import run_bass_kernel_spmd

    x = np.asarray(x, dtype=np.float32)
    if "nc" not in _CACHE:
        _CACHE["nc"] = _build_nc()
    nc = _CACHE["nc"]
    trace = os.environ.get("BISPEC_TRACE", "0") == "1"
    res = run_bass_kernel_spmd(
        nc, _in_maps(x), core_ids=list(range(NCORES)), trace=trace
    )
    _CACHE["last_exec_time_ns"] = res.exec_time_ns
    _CACHE["last_res"] = res
    return _assemble(res.results)
